# revision 30
# baseline (speedup 1.0000x reference)
"""Bayer kernel-prediction demosaic network on 8 Trainium2 NeuronCores.

Row-interleaved activation layout: each conv layer's output lives as
[128 partitions, row-pairs * GW]; partitions 0:64 hold channel c of EVEN
rows, 64:128 hold channel c of ODD rows of each row-pair.
  - conv1..3: 12 matmuls per 512-px tile (3 K=128 dy-pairs + 3 K=64
    singles per row parity), two concurrent 64-col streams via column
    groups, one [128, 512] PSUM tile; eviction is ONE relu+bias ACT
    per tile (no PSUM-half merge, no shifted-duplicate copies)
  - conv4 (64->490) runs per (row-pair, m-block of <=128): 10 matmuls
    (3 K=128 dy-pairs per parity + 1 K=128 column-shift pair from a
    dup buffer + 1 K=64 single per parity, the two singles on disjoint
    row/column quadrants); 2-row PSUM batches evicted by one exp
    ACT -> E
  - den/num group reductions contract 490 channels -> 8 groups, 4-way
    column-packed at tile positions (0,0)/(0,32)/(0,64)/(0,96) ->
    packed reciprocal / final multiply
  - emission is software-pipelined across the 6 row blocks so TensorE
    always has queued work (keeps the PE HAM clock gate warm)
Host does phase extraction, weight re-layout, patch-tensor build,
sharding w/ halo, and the final pixel-shuffle assembly.
"""

import sys

sys.path.insert(0, "/opt/trn_rl_repo")

import numpy as np
import ml_dtypes

# ---------------- geometry constants ----------------
KS = 7
K2 = 49
BS = 2
H = W = 768
QH = QW = 384          # quarter-res
KR_TOT = 374           # valid kernel rows/cols
BANDS = 4              # bands per batch -> 8 cores
KR = 96                # kernel rows computed per core (94/92 valid)
RB = 16                # kernel rows per block
SRB = 8                # kernel rows per apply sub-block
NBLK = KR // RB
GW = 386               # conv grid width
EW = 376               # apply/kernel grid width (374 valid + 2)
XW = 388               # x slab width (384 data + 4 zero)
XR = 106               # x slab rows (96 + 10)
NP0, NP1, NP2, NP3 = 12, 11, 10, 9     # row-pairs per conv output/block
NSB = 3                # superblocks (2 row blocks each) for conv0/conv1
P0M, P1M, P2M = 20, 19, 18     # merged conv row-pairs per superblock
F0M, F1M, F2M = P0M * 386, P1M * 386, P2M * 386
F0, F1, F2, F3 = NP0 * GW, NP1 * GW, NP2 * GW, NP3 * GW
FE = SRB * EW          # flat apply pixels per sub-block (3008)
OUTF = KR * EW
MBLK = [0, 128, 256, 384, 490]     # channel block boundaries
# plane (x-slab channel) feeding each 49-tap chunk of the 490 kernels:
# x channels: 0=g0 1=b 2=r 3=g1 ; chunks: 3x red, 3x blue, (g0,g1)x2
CHUNK_PLANE = [2, 2, 2, 1, 1, 1, 0, 3, 0, 3]
# 49-chunk -> output group (greens pair up)
CHUNK_GROUP = [0, 1, 2, 3, 4, 5, 6, 6, 7, 7]
# den/num column chunks, grouped into packed PSUM tiles
CHUNKS = [(o, min(512, FE - o)) for o in range(0, FE, 512)]
CGROUPS = [CHUNKS[0:4], CHUNKS[4:6]]
MULT_H = 2656          # DVE/GpSimd split point for the E*Prep multiply

TRACE = False          # set True (module attr) to profile the run
LAST_EXEC_NS = None
LAST_RESULTS = None

_cache = {}


def _build():
    import concourse.bass as bass
    import concourse.bacc as bacc
    import concourse.mybir as mybir
    import concourse.tile as tile

    f32 = mybir.dt.float32
    f16 = mybir.dt.float16
    bf16 = mybir.dt.bfloat16
    AF = mybir.ActivationFunctionType

    nc = bacc.Bacc("TRN2", target_bir_lowering=False, debug=False,
                   enable_asserts=False)

    xs = nc.dram_tensor("xs", [4, XR, XW], f16, kind="ExternalInput")
    xi = nc.dram_tensor("xi", [100, NSB, F0M], f16, kind="ExternalInput")
    xg = nc.dram_tensor("xg", [490, KR, EW], bf16, kind="ExternalInput")
    w0 = nc.dram_tensor("w0", [100, 64], f16, kind="ExternalInput")
    wpe = nc.dram_tensor("wpe", [128, 9, 64], f16, kind="ExternalInput")
    wpo = nc.dram_tensor("wpo", [128, 9, 64], f16, kind="ExternalInput")
    wsng = nc.dram_tensor("wsng", [128, 9, 64], f16, kind="ExternalInput")
    w4pe = nc.dram_tensor("w4pe", [128, 3, 490], f16, kind="ExternalInput")
    w4po = nc.dram_tensor("w4po", [128, 3, 490], f16, kind="ExternalInput")
    w4d = nc.dram_tensor("w4d", [128, 2, 490], f16, kind="ExternalInput")
    w4s = nc.dram_tensor("w4s", [128, 490], f16, kind="ExternalInput")
    w4x = nc.dram_tensor("w4x", [128, 2, 490], f16, kind="ExternalInput")
    b03 = nc.dram_tensor("b03", [128, 4], f32, kind="ExternalInput")
    b4 = nc.dram_tensor("b4", [128, 4], f32, kind="ExternalInput")
    gm = nc.dram_tensor("gm", [128, 4, 8], bf16, kind="ExternalInput")
    out = nc.dram_tensor("out", [8, OUTF], f32, kind="ExternalOutput")

    def ntiles(total, tsz=512):
        o = 0
        while o < total:
            n = min(tsz, total - o)
            yield o, n
            o += n

    with tile.TileContext(nc) as tc:
        with (
            tc.tile_pool(name="wts", bufs=1) as wts,
            tc.tile_pool(name="xp", bufs=1) as xp,
            tc.tile_pool(name="lp", bufs=1) as lp,
            tc.tile_pool(name="pp", bufs=2) as pp,
            tc.tile_pool(name="ep", bufs=1) as ep,
            tc.tile_pool(name="smp", bufs=2) as smp,
            tc.tile_pool(name="pscv", bufs=2, space="PSUM") as pscv,
            tc.tile_pool(name="ps4p", bufs=2, space="PSUM") as ps4p,
            tc.tile_pool(name="psa", bufs=2, space="PSUM") as psa,
        ):
            w0_sb = wts.tile([100, 64], f16)
            wpe_sb = wts.tile([128, 9, 64], f16)
            wpo_sb = wts.tile([128, 9, 64], f16)
            wsng_sb = wts.tile([128, 9, 64], f16)
            b03_sb = wts.tile([128, 4], f32)
            b4_sb = wts.tile([128, 4], f32)
            gm_sb = wts.tile([128, 4, 8], bf16)
            w4pe_sb = wts.tile([128, 3, 490], f16)
            w4po_sb = wts.tile([128, 3, 490], f16)
            w4d_sb = wts.tile([128, 2, 490], f16)
            w4s_sb = wts.tile([128, 490], f16)
            w4x_sb = wts.tile([128, 2, 490], f16)
            st = {}

            def emit_weights(group):
                for dst, src in group:
                    nc.sync.dma_start(dst[:], src.ap())

            def emit_x36(sb, nchunks=4):
                x36t = xp.tile([100, F0M], f16, tag="x36", bufs=1,
                               name=f"x36_{sb}")
                cuts = [F0M * i // nchunks for i in range(nchunks + 1)]
                for c0, c1 in zip(cuts, cuts[1:]):
                    src = bass.AP(xi, sb * F0M + c0,
                                  [[NSB * F0M, 100], [1, c1 - c0]])
                    nc.sync.dma_start(x36t[0:100, c0:c1], src)
                st[('x36m', sb)] = x36t

            def emit_prep(b):
                R = b * RB
                preps = []
                HF = FE // 2
                for s in range(2):
                    P = pp.tile([128, 4, FE], bf16, tag="prep", bufs=2,
                                name=f"prep{b}_{s}")
                    for m in range(4):
                        mm = MBLK[m + 1] - MBLK[m]
                        for fo in (0, HF):
                            src = bass.AP(
                                xg,
                                MBLK[m] * KR * EW + (R + s * SRB) * EW + fo,
                                [[KR * EW, mm], [1, HF]])
                            nc.sync.dma_start(P[0:mm, m, fo:fo + HF], src)
                    preps.append(P)
                st[('prep', b)] = preps

            def emit_conv0(sb):
                x36t = st.pop(('x36m', sb))
                L0 = lp.tile([128, F0M + 2], f16, tag="l0", bufs=1,
                             name=f"L0_{sb}")
                for o, n in ntiles(F0M):
                    ps = pscv.tile([128, 512], f32, tag="pscv",
                                   name=f"ps0_{sb}_{o}")
                    nc.tensor.matmul(ps[0:64, 0:n], w0_sb[0:36, :],
                                     x36t[0:36, o:o + n], start=True,
                                     stop=True, skip_group_check=True)
                    nc.tensor.matmul(ps[64:128, 0:n], w0_sb[64:100, :],
                                     x36t[64:100, o:o + n], start=True,
                                     stop=True, skip_group_check=True)
                    nc.scalar.activation(L0[0:128, o:o + n], ps[0:128, 0:n],
                                         AF.Relu, bias=b03_sb[:, 0:1])
                st[('L0m', sb)] = L0

            def emit_convi(key, li, Fi, tg, nb, Lp, ioff=0):
                Li = lp.tile([128, Fi + 2], f16, tag=tg, bufs=nb,
                             name=f"L{li + 1}_{key}")
                for o, n in ntiles(Fi):
                    oi = ioff + o
                    ps = pscv.tile([128, 512], f32, tag="pscv",
                                   name=f"ps{li + 1}_{key}_{o}")
                    for dx in range(3):
                        nc.tensor.matmul(ps[0:64, 0:n],
                                         wpe_sb[:, 3 * li + dx, :],
                                         Lp[0:128, oi + dx:oi + dx + n],
                                         start=(dx == 0), stop=False,
                                         skip_group_check=True)
                        nc.tensor.matmul(
                            ps[64:128, 0:n], wpo_sb[:, 3 * li + dx, :],
                            Lp[0:128, oi + GW + dx:oi + GW + dx + n],
                            start=(dx == 0), stop=False,
                            skip_group_check=True)
                    for dx in range(3):
                        nc.tensor.matmul(
                            ps[0:64, 0:n], wsng_sb[0:64, 3 * li + dx, :],
                            Lp[0:64, oi + GW + dx:oi + GW + dx + n],
                            start=False, stop=(dx == 2),
                            skip_group_check=True)
                        nc.tensor.matmul(
                            ps[64:128, 0:n], wsng_sb[64:128, 3 * li + dx, :],
                            Lp[64:128, oi + dx:oi + dx + n],
                            start=False, stop=(dx == 2),
                            skip_group_check=True)
                    nc.scalar.activation(Li[0:128, o:o + n], ps[0:128, 0:n],
                                         AF.Relu,
                                         bias=b03_sb[:, li + 1:li + 2])
                return Li

            def emit_conv1m(sb):
                st[('L1m', sb)] = emit_convi(f"m{sb}", 0, F1M, "l1", 1,
                                             st.pop(('L0m', sb)))

            def emit_conv2m(sb):
                st[('L2m', sb)] = emit_convi(f"c2m{sb}", 1, F2M, "l2", 1,
                                             st[('L1m', sb)])

            def emit_conv3(b):
                st[('L', b)] = emit_convi(b, 2, F3, "l3", 2,
                                          st[('L2m', b // 2)],
                                          ioff=(b % 2) * 8 * GW)

            def emit_ldup(b):
                L3 = st[('L', b)]
                de = lp.tile([128, F3 + 2], f16, tag="ldupe", bufs=1,
                             name=f"de_{b}")
                do = lp.tile([128, F3 + 2], f16, tag="ldupo", bufs=1,
                             name=f"do_{b}")
                for c0, c1 in ((0, 2 * GW + 378), (2 * GW + 378, 5 * GW),
                               (5 * GW, F3)):
                    nc.sync.dma_start(de[0:64, c0:c1], L3[0:64, c0 + 1:c1 + 1])
                    nc.sync.dma_start(de[64:128, c0:c1],
                                      L3[0:64, c0 + 2:c1 + 2])
                    nc.sync.dma_start(do[0:64, c0:c1],
                                      L3[64:128, c0 + 1:c1 + 1])
                    nc.sync.dma_start(do[64:128, c0:c1],
                                      L3[64:128, c0 + 2:c1 + 2])
                st[('ldup', b)] = (de, do)

            def conv4_sub(b, s):
                L3 = st[('L', b)]
                de, do = st[('ldup', b)]
                E = ep.tile([128, 4, FE], bf16, tag=f"e{s}", bufs=1,
                            name=f"E{b}_{s}")
                for rp in range(4):
                    rho = s * 4 + rp
                    oe = rho * GW
                    oo = (rho + 1) * GW
                    for m in range(4):
                        mm = MBLK[m + 1] - MBLK[m]
                        ms = slice(MBLK[m], MBLK[m + 1])
                        ps4 = ps4p.tile([128, 2, 512], f32, tag="ps4",
                                        name=f"ps4_{b}_{rho}_{m}")
                        # dy-pairs (K=128)
                        for dx in range(3):
                            nc.tensor.matmul(
                                ps4[0:mm, 0, 0:EW], w4pe_sb[:, dx, ms],
                                L3[0:128, oe + dx:oe + dx + EW],
                                start=(dx == 0), stop=False,
                                skip_group_check=True)
                            nc.tensor.matmul(
                                ps4[0:mm, 1, 0:EW], w4po_sb[:, dx, ms],
                                L3[0:128, oo + dx:oo + dx + EW],
                                start=(dx == 0), stop=False,
                                skip_group_check=True)
                        if s == 0 and (rp == 0 or (b == 0 and rp == 1)):
                            # no-dup form: 3 singles per parity, so the
                            # first rows of a block never wait on the
                            # ldup DMA chain
                            for j, wt in ((0, w4s_sb[0:64, ms]),
                                          (1, w4x_sb[0:64, 0, ms]),
                                          (2, w4x_sb[0:64, 1, ms])):
                                nc.tensor.matmul(
                                    ps4[0:mm, 0, 0:EW], wt,
                                    L3[0:64, oo + j:oo + j + EW],
                                    start=False, stop=(j == 2),
                                    skip_group_check=True)
                            for j, wt in ((0, w4s_sb[64:128, ms]),
                                          (1, w4x_sb[64:128, 0, ms]),
                                          (2, w4x_sb[64:128, 1, ms])):
                                nc.tensor.matmul(
                                    ps4[0:mm, 1, 0:EW], wt,
                                    L3[64:128, oe + j:oe + j + EW],
                                    start=False, stop=(j == 2),
                                    skip_group_check=True)
                            nc.scalar.activation(
                                E[0:mm, m, 2 * rp * EW:(2 * rp + 2) * EW],
                                ps4[0:mm, :, 0:EW], AF.Exp,
                                bias=b4_sb[0:mm, m:m + 1])
                            continue
                        # column-shift dup pair (K=128)
                        nc.tensor.matmul(ps4[0:mm, 0, 0:EW],
                                         w4d_sb[:, 0, ms],
                                         de[0:128, oo:oo + EW],
                                         start=False, stop=False,
                                         skip_group_check=True)
                        nc.tensor.matmul(ps4[0:mm, 1, 0:EW],
                                         w4d_sb[:, 1, ms],
                                         do[0:128, oe:oe + EW],
                                         start=False, stop=False,
                                         skip_group_check=True)
                        # the two K=64 singles occupy disjoint row groups
                        nc.tensor.matmul(ps4[0:mm, 0, 0:EW],
                                         w4s_sb[0:64, ms],
                                         L3[0:64, oo:oo + EW],
                                         start=False, stop=True,
                                         skip_group_check=True)
                        nc.tensor.matmul(ps4[0:mm, 1, 0:EW],
                                         w4s_sb[64:128, ms],
                                         L3[64:128, oe:oe + EW],
                                         start=False, stop=True,
                                         skip_group_check=True)
                        nc.scalar.activation(
                            E[0:mm, m, 2 * rp * EW:(2 * rp + 2) * EW],
                            ps4[0:mm, :, 0:EW], AF.Exp,
                            bias=b4_sb[0:mm, m:m + 1])
                return E

            def grp_tile(tag, b, s, ti, grp, E):
                nd = psa.tile([128, 512], f32, tag="psa",
                              name=f"{tag}{b}_{s}_{ti}")
                for m in range(4):
                    mm = MBLK[m + 1] - MBLK[m]
                    for j, (o, n) in enumerate(grp):
                        nc.tensor.matmul(nd[32 * j:32 * j + 8, 0:n],
                                         gm_sb[0:mm, m, :],
                                         E[0:mm, m, o:o + n],
                                         start=(m == 0), stop=(m == 3),
                                         tile_position=(0, 32 * j),
                                         skip_group_check=True)
                return nd

            def den_sub(b, s, E):
                recs = []
                for ti, grp in enumerate(CGROUPS):
                    nd = grp_tile("den", b, s, ti, grp, E)
                    Pn = 32 * (len(grp) - 1) + 8
                    rec = smp.tile([128, 512], f32, tag="rec", bufs=4,
                                   name=f"rec{b}_{s}_{ti}")
                    nc.vector.reciprocal_approx_fast(rec[0:Pn, :],
                                                     nd[0:Pn, 0:512])
                    recs.append(rec)
                return recs

            def mult_sub(b, s, E):
                Prep = st[('prep', b)][s]
                for m in range(4):
                    mm = MBLK[m + 1] - MBLK[m]
                    nc.vector.tensor_mul(E[0:mm, m, 0:MULT_H],
                                         E[0:mm, m, 0:MULT_H],
                                         Prep[0:mm, m, 0:MULT_H])
                    nc.gpsimd.tensor_mul(E[0:mm, m, MULT_H:FE],
                                         E[0:mm, m, MULT_H:FE],
                                         Prep[0:mm, m, MULT_H:FE])

            def num_sub(b, s, E, recs):
                base = (b * RB + s * SRB) * EW
                for ti, grp in enumerate(CGROUPS):
                    nd = grp_tile("num", b, s, ti, grp, E)
                    Pn = 32 * (len(grp) - 1) + 8
                    res = smp.tile([128, 512], f32, tag="res", bufs=2,
                                   name=f"res{b}_{s}_{ti}")
                    nc.vector.tensor_mul(res[0:Pn, :], nd[0:Pn, 0:512],
                                         recs[ti][0:Pn, :])
                    for j, (o, n) in enumerate(grp):
                        nc.sync.dma_start(
                            out.ap()[0:8, base + o:base + o + n],
                            res[32 * j:32 * j + 8, 0:n])

            emit_weights([(w0_sb, w0)])
            emit_x36(0, nchunks=8)
            emit_weights([(b03_sb, b03), (wpe_sb, wpe), (wpo_sb, wpo),
                          (wsng_sb, wsng), (w4pe_sb, w4pe), (w4po_sb, w4po),
                          (w4d_sb, w4d), (w4s_sb, w4s), (w4x_sb, w4x),
                          (b4_sb, b4), (gm_sb, gm)])
            emit_conv0(0)
            emit_prep(0)
            emit_conv1m(0)
            emit_conv2m(0)
            emit_conv3(0)
            emit_conv3(1)
            for b in range(NBLK):
                k = b % 2
                sb = b // 2
                emit_ldup(b)
                if k == 0 and sb + 1 < NSB:
                    emit_x36(sb + 1)
                if b + 1 < NBLK:
                    emit_prep(b + 1)
                E0 = conv4_sub(b, 0)
                recs0 = den_sub(b, 0, E0)
                E1 = conv4_sub(b, 1)
                mult_sub(b, 0, E0)
                recs1 = den_sub(b, 1, E1)
                mult_sub(b, 1, E1)
                if k == 0 and sb + 1 < NSB:
                    emit_conv0(sb + 1)
                elif k == 1 and b + 2 < NBLK:
                    emit_conv3(b + 1)
                elif b == 4:
                    emit_conv3(5)
                num_sub(b, 0, E0, recs0)
                if k == 0 and sb + 1 < NSB:
                    emit_conv1m(sb + 1)
                elif b == 1:
                    emit_conv3(3)
                num_sub(b, 1, E1, recs1)
                if k == 0 and sb + 1 < NSB:
                    emit_conv2m(sb + 1)

    nc.compile()
    return nc


def _host_prep(inputs):
    mosaic = np.asarray(inputs["mosaic"], dtype=np.float32)
    gray = mosaic.sum(axis=1)                       # [2, 768, 768]
    g0 = gray[:, 0::2, 0::2]
    b_ = gray[:, 1::2, 0::2]
    r = gray[:, 0::2, 1::2]
    g1 = gray[:, 1::2, 1::2]
    x4 = np.stack([g0, b_, r, g1], axis=1)          # [2, 4, 384, 384]
    xpad = np.zeros((BS, 4, QH + 4, XW), dtype=np.float32)
    xpad[:, :, :QH, :QW] = x4

    W0 = np.asarray(inputs["W0"], np.float32)
    w0v = np.zeros((100, 64), np.float32)
    w0flat = W0.transpose(2, 3, 1, 0).reshape(36, 64)
    w0v[0:36] = w0flat
    w0v[64:100] = w0flat

    wpe = np.zeros((128, 9, 64), np.float32)
    wpo = np.zeros((128, 9, 64), np.float32)
    wsng = np.zeros((128, 9, 64), np.float32)
    for li, wname in enumerate(("W1", "W2", "W3")):
        Wi = np.asarray(inputs[wname], np.float32)   # [64out, 64in, 3, 3]
        for dx in range(3):
            wpe[0:64, 3 * li + dx, :] = Wi[:, :, 0, dx].T
            wpe[64:128, 3 * li + dx, :] = Wi[:, :, 1, dx].T
            wpo[0:64, 3 * li + dx, :] = Wi[:, :, 1, dx].T
            wpo[64:128, 3 * li + dx, :] = Wi[:, :, 2, dx].T
            wsng[0:64, 3 * li + dx, :] = Wi[:, :, 2, dx].T
            wsng[64:128, 3 * li + dx, :] = Wi[:, :, 0, dx].T

    W4 = np.asarray(inputs["W4"], np.float32)        # [490, 64, 3, 3]
    w4pe = np.zeros((128, 3, 490), np.float32)
    w4po = np.zeros((128, 3, 490), np.float32)
    w4d = np.zeros((128, 2, 490), np.float32)
    w4s = np.zeros((128, 490), np.float32)
    for dx in range(3):
        w4pe[0:64, dx, :] = W4[:, :, 0, dx].T
        w4pe[64:128, dx, :] = W4[:, :, 1, dx].T
        w4po[0:64, dx, :] = W4[:, :, 1, dx].T
        w4po[64:128, dx, :] = W4[:, :, 2, dx].T
    w4d[0:64, 0, :] = W4[:, :, 2, 1].T
    w4d[64:128, 0, :] = W4[:, :, 2, 2].T
    w4d[0:64, 1, :] = W4[:, :, 0, 1].T
    w4d[64:128, 1, :] = W4[:, :, 0, 2].T
    w4s[0:64, :] = W4[:, :, 2, 0].T
    w4s[64:128, :] = W4[:, :, 0, 0].T
    w4x = np.zeros((128, 2, 490), np.float32)
    w4x[0:64, 0, :] = W4[:, :, 2, 1].T
    w4x[0:64, 1, :] = W4[:, :, 2, 2].T
    w4x[64:128, 0, :] = W4[:, :, 0, 1].T
    w4x[64:128, 1, :] = W4[:, :, 0, 2].T

    b03 = np.zeros((128, 4), np.float32)
    for i in range(4):
        bi = np.asarray(inputs[f"b{i}"], np.float32)
        b03[0:64, i] = bi
        b03[64:128, i] = bi
    b4v = np.asarray(inputs["b4"], np.float32)
    b4p = np.zeros((128, 4), np.float32)
    for c in range(490):
        b4p[c % 128, c // 128] = b4v[c]

    gmk = np.zeros((128, 4, 8), ml_dtypes.bfloat16)
    for c in range(490):
        gmk[c % 128, c // 128, CHUNK_GROUP[c // 49]] = 1

    xpad_bf = xpad.astype(ml_dtypes.bfloat16)
    wcast = {
        "w0": w0v.astype(np.float16),
        "wpe": wpe.astype(np.float16),
        "wpo": wpo.astype(np.float16),
        "wsng": wsng.astype(np.float16),
        "w4pe": w4pe.astype(np.float16),
        "w4po": w4po.astype(np.float16),
        "w4d": w4d.astype(np.float16),
        "w4s": w4s.astype(np.float16),
        "w4x": w4x.astype(np.float16),
        "b03": b03, "b4": b4p, "gm": gmk,
    }
    in_maps = []
    for b in range(BS):
        for band in range(BANDS):
            r0 = band * 94
            slab = np.zeros((4, XR, XW), np.float16)
            hi = min(QH, r0 + XR)
            slab[:, 0:hi - r0, :] = xpad[b, :, r0:hi, :].astype(np.float16)
            # shifted-plane (im2col) tensor for the kernel-apply patches:
            # xg[49*j + 7*dy + dx, jr, jc] = plane_j[r0 + jr + 2 + dy, jc + 2 + dx]
            xgp = np.empty((490, KR, EW), ml_dtypes.bfloat16)
            for j in range(10):
                pl = xpad_bf[b, CHUNK_PLANE[j]]
                for dy in range(KS):
                    for dx in range(KS):
                        c = 49 * j + 7 * dy + dx
                        xgp[c] = pl[r0 + 2 + dy: r0 + 2 + dy + KR,
                                    2 + dx: 2 + dx + EW]
            xi = np.zeros((100, NSB, P0M * GW), np.float16)
            for sb in range(NSB):
                R = sb * 2 * RB
                for dy in range(3):
                    for dx in range(3):
                        for ch in range(4):
                            p = 4 * (3 * dy + dx) + ch
                            ev = slab[ch, R + dy:R + dy + 2 * P0M:2,
                                      dx:dx + GW]
                            od = slab[ch, R + 1 + dy:R + 1 + dy + 2 * P0M:2,
                                      dx:dx + GW]
                            xi[p, sb] = ev.reshape(-1)
                            xi[64 + p, sb] = od.reshape(-1)
            im = {"xs": slab, "xg": xgp, "xi": xi}
            im.update(wcast)
            in_maps.append(im)
    aux = {"g0": g0, "b_": b_, "r": r, "g1": g1}
    return in_maps, aux


def _assemble(results, aux):
    full = np.empty((BS, 3, 2 * KR_TOT, 2 * KR_TOT), np.float32)
    # quarter-res computed planes [8, 374, 374] per batch
    for b in range(BS):
        qs = []
        for band in range(BANDS):
            core = b * BANDS + band
            o = results[core]["out"].reshape(8, KR, EW)
            nvalid = min(94, KR_TOT - band * 94)
            qs.append(o[:, :nvalid, :KR_TOT])
        q = np.concatenate(qs, axis=1)               # [8, 374, 374]
        crop = (slice(5, 5 + KR_TOT), slice(5, 5 + KR_TOT))
        r_pass = aux["r"][b][crop]
        b_pass = aux["b_"][b][crop]
        g0_pass = aux["g0"][b][crop]
        g1_pass = aux["g1"][b][crop]
        # red
        full[b, 0, 0::2, 0::2] = q[0]
        full[b, 0, 0::2, 1::2] = r_pass
        full[b, 0, 1::2, 0::2] = q[1]
        full[b, 0, 1::2, 1::2] = q[2]
        # green
        full[b, 1, 0::2, 0::2] = g0_pass
        full[b, 1, 0::2, 1::2] = q[6]
        full[b, 1, 1::2, 0::2] = q[7]
        full[b, 1, 1::2, 1::2] = g1_pass
        # blue
        full[b, 2, 0::2, 0::2] = q[3]
        full[b, 2, 0::2, 1::2] = q[4]
        full[b, 2, 1::2, 0::2] = b_pass
        full[b, 2, 1::2, 1::2] = q[5]
    return full


def kernel(**inputs):
    global LAST_EXEC_NS, LAST_RESULTS
    from concourse.bass_utils import run_bass_kernel_spmd

    if "nc" not in _cache:
        _cache["nc"] = _build()
    nc = _cache["nc"]

    in_maps, aux = _host_prep(inputs)
    kw = {}
    if TRACE:
        kw["trace"] = True
    res = run_bass_kernel_spmd(nc, in_maps, core_ids=list(range(8)), **kw)
    LAST_EXEC_NS = res.exec_time_ns
    LAST_RESULTS = res
    return _assemble(res.results, aux)


# revision 31
# speedup vs baseline: 1.0291x; 1.0291x over previous
"""Bayer kernel-prediction demosaic network on 8 Trainium2 NeuronCores.

Row-interleaved activation layout: each conv layer's output lives as
[128 partitions, row-pairs * GW]; partitions 0:64 hold channel c of EVEN
rows, 64:128 hold channel c of ODD rows of each row-pair.
  - conv1..3: 12 matmuls per 512-px tile (3 K=128 dy-pairs + 3 K=64
    singles per row parity), two concurrent 64-col streams via column
    groups, one [128, 512] PSUM tile; eviction is ONE relu+bias ACT
    per tile (no PSUM-half merge, no shifted-duplicate copies)
  - conv4 (64->490) runs per (row-pair, m-block of <=128): 10 matmuls
    (3 K=128 dy-pairs per parity + 1 K=128 column-shift pair from a
    dup buffer + 1 K=64 single per parity, the two singles on disjoint
    row/column quadrants); 2-row PSUM batches evicted by one exp
    ACT -> E
  - den/num group reductions contract 490 channels -> 8 groups, 4-way
    column-packed at tile positions (0,0)/(0,32)/(0,64)/(0,96) ->
    packed reciprocal / final multiply
  - emission is software-pipelined across the 6 row blocks so TensorE
    always has queued work (keeps the PE HAM clock gate warm)
Host does phase extraction, weight re-layout, patch-tensor build,
sharding w/ halo, and the final pixel-shuffle assembly.
"""

import sys

sys.path.insert(0, "/opt/trn_rl_repo")

import numpy as np
import ml_dtypes

# ---------------- geometry constants ----------------
KS = 7
K2 = 49
BS = 2
H = W = 768
QH = QW = 384          # quarter-res
KR_TOT = 374           # valid kernel rows/cols
BANDS = 4              # bands per batch -> 8 cores
KR = 96                # kernel rows computed per core (94/92 valid)
RB = 16                # kernel rows per block
SRB = 8                # kernel rows per apply sub-block
NBLK = KR // RB
GW = 386               # conv grid width
EW = 376               # apply/kernel grid width (374 valid + 2)
XW = 388               # x slab width (384 data + 4 zero)
XR = 106               # x slab rows (96 + 10)
NP0, NP1, NP2, NP3 = 12, 11, 10, 9     # row-pairs per conv output/block
NSB = 3                # superblocks (2 row blocks each) for conv0/conv1
P0M, P1M = 20, 19      # merged conv0/conv1 row-pairs per superblock
F0M, F1M = P0M * 386, P1M * 386
F0, F1, F2, F3 = NP0 * GW, NP1 * GW, NP2 * GW, NP3 * GW
FE = SRB * EW          # flat apply pixels per sub-block (3008)
OUTF = KR * EW
MBLK = [0, 128, 256, 384, 490]     # channel block boundaries
# plane (x-slab channel) feeding each 49-tap chunk of the 490 kernels:
# x channels: 0=g0 1=b 2=r 3=g1 ; chunks: 3x red, 3x blue, (g0,g1)x2
CHUNK_PLANE = [2, 2, 2, 1, 1, 1, 0, 3, 0, 3]
# 49-chunk -> output group (greens pair up)
CHUNK_GROUP = [0, 1, 2, 3, 4, 5, 6, 6, 7, 7]
# den/num column chunks, grouped into packed PSUM tiles
CHUNKS = [(o, min(512, FE - o)) for o in range(0, FE, 512)]
CGROUPS = [CHUNKS[0:4], CHUNKS[4:6]]
MULT_H = 2656          # DVE/GpSimd split point for the E*Prep multiply

TRACE = False          # set True (module attr) to profile the run
LAST_EXEC_NS = None
LAST_RESULTS = None

_cache = {}


def _build():
    import concourse.bass as bass
    import concourse.bacc as bacc
    import concourse.mybir as mybir
    import concourse.tile as tile

    f32 = mybir.dt.float32
    f16 = mybir.dt.float16
    bf16 = mybir.dt.bfloat16
    AF = mybir.ActivationFunctionType

    nc = bacc.Bacc("TRN2", target_bir_lowering=False, debug=False,
                   enable_asserts=False)

    xs = nc.dram_tensor("xs", [4, XR, XW], f16, kind="ExternalInput")
    xi = nc.dram_tensor("xi", [100, NSB, F0M], f16, kind="ExternalInput")
    xg = nc.dram_tensor("xg", [490, KR, EW], bf16, kind="ExternalInput")
    w0 = nc.dram_tensor("w0", [100, 64], f16, kind="ExternalInput")
    wpe = nc.dram_tensor("wpe", [128, 9, 64], f16, kind="ExternalInput")
    wpo = nc.dram_tensor("wpo", [128, 9, 64], f16, kind="ExternalInput")
    wsng = nc.dram_tensor("wsng", [128, 9, 64], f16, kind="ExternalInput")
    w4pe = nc.dram_tensor("w4pe", [128, 3, 490], f16, kind="ExternalInput")
    w4po = nc.dram_tensor("w4po", [128, 3, 490], f16, kind="ExternalInput")
    w4d = nc.dram_tensor("w4d", [128, 2, 490], f16, kind="ExternalInput")
    w4s = nc.dram_tensor("w4s", [128, 490], f16, kind="ExternalInput")
    w4x = nc.dram_tensor("w4x", [128, 2, 490], f16, kind="ExternalInput")
    b03 = nc.dram_tensor("b03", [128, 4], f32, kind="ExternalInput")
    b4 = nc.dram_tensor("b4", [128, 4], f32, kind="ExternalInput")
    gm = nc.dram_tensor("gm", [128, 4, 8], bf16, kind="ExternalInput")
    out = nc.dram_tensor("out", [8, OUTF], f32, kind="ExternalOutput")

    def ntiles(total, tsz=512):
        o = 0
        while o < total:
            n = min(tsz, total - o)
            yield o, n
            o += n

    with tile.TileContext(nc) as tc:
        with (
            tc.tile_pool(name="wts", bufs=1) as wts,
            tc.tile_pool(name="xp", bufs=1) as xp,
            tc.tile_pool(name="lp", bufs=1) as lp,
            tc.tile_pool(name="pp", bufs=2) as pp,
            tc.tile_pool(name="ep", bufs=1) as ep,
            tc.tile_pool(name="smp", bufs=2) as smp,
            tc.tile_pool(name="pscv", bufs=2, space="PSUM") as pscv,
            tc.tile_pool(name="ps4p", bufs=2, space="PSUM") as ps4p,
            tc.tile_pool(name="psa", bufs=2, space="PSUM") as psa,
        ):
            w0_sb = wts.tile([100, 64], f16)
            wpe_sb = wts.tile([128, 9, 64], f16)
            wpo_sb = wts.tile([128, 9, 64], f16)
            wsng_sb = wts.tile([128, 9, 64], f16)
            b03_sb = wts.tile([128, 4], f32)
            b4_sb = wts.tile([128, 4], f32)
            gm_sb = wts.tile([128, 4, 8], bf16)
            w4pe_sb = wts.tile([128, 3, 490], f16)
            w4po_sb = wts.tile([128, 3, 490], f16)
            w4d_sb = wts.tile([128, 2, 490], f16)
            w4s_sb = wts.tile([128, 490], f16)
            w4x_sb = wts.tile([128, 2, 490], f16)
            st = {}

            def emit_weights(group):
                for dst, src in group:
                    nc.sync.dma_start(dst[:], src.ap())

            def emit_x36(sb, nchunks=4):
                x36t = xp.tile([100, F0M], f16, tag="x36", bufs=1,
                               name=f"x36_{sb}")
                cuts = [F0M * i // nchunks for i in range(nchunks + 1)]
                for c0, c1 in zip(cuts, cuts[1:]):
                    src = bass.AP(xi, sb * F0M + c0,
                                  [[NSB * F0M, 100], [1, c1 - c0]])
                    nc.sync.dma_start(x36t[0:100, c0:c1], src)
                st[('x36m', sb)] = x36t

            def emit_prep(b):
                R = b * RB
                preps = []
                HF = FE // 2
                for s in range(2):
                    P = pp.tile([128, 4, FE], bf16, tag="prep", bufs=2,
                                name=f"prep{b}_{s}")
                    for m in range(4):
                        mm = MBLK[m + 1] - MBLK[m]
                        for fo in (0, HF):
                            src = bass.AP(
                                xg,
                                MBLK[m] * KR * EW + (R + s * SRB) * EW + fo,
                                [[KR * EW, mm], [1, HF]])
                            nc.sync.dma_start(P[0:mm, m, fo:fo + HF], src)
                    preps.append(P)
                st[('prep', b)] = preps

            def emit_conv0(sb):
                x36t = st.pop(('x36m', sb))
                L0 = lp.tile([128, F0M + 2], f16, tag="l0", bufs=1,
                             name=f"L0_{sb}")
                for o, n in ntiles(F0M):
                    ps = pscv.tile([128, 512], f32, tag="pscv",
                                   name=f"ps0_{sb}_{o}")
                    nc.tensor.matmul(ps[0:64, 0:n], w0_sb[0:36, :],
                                     x36t[0:36, o:o + n], start=True,
                                     stop=True, skip_group_check=True)
                    nc.tensor.matmul(ps[64:128, 0:n], w0_sb[64:100, :],
                                     x36t[64:100, o:o + n], start=True,
                                     stop=True, skip_group_check=True)
                    nc.scalar.activation(L0[0:128, o:o + n], ps[0:128, 0:n],
                                         AF.Relu, bias=b03_sb[:, 0:1])
                st[('L0m', sb)] = L0

            def emit_convi(key, li, Fi, tg, nb, Lp, ioff=0):
                Li = lp.tile([128, Fi + 2], f16, tag=tg, bufs=nb,
                             name=f"L{li + 1}_{key}")
                for o, n in ntiles(Fi):
                    oi = ioff + o
                    ps = pscv.tile([128, 512], f32, tag="pscv",
                                   name=f"ps{li + 1}_{key}_{o}")
                    for dx in range(3):
                        nc.tensor.matmul(ps[0:64, 0:n],
                                         wpe_sb[:, 3 * li + dx, :],
                                         Lp[0:128, oi + dx:oi + dx + n],
                                         start=(dx == 0), stop=False,
                                         skip_group_check=True)
                        nc.tensor.matmul(
                            ps[64:128, 0:n], wpo_sb[:, 3 * li + dx, :],
                            Lp[0:128, oi + GW + dx:oi + GW + dx + n],
                            start=(dx == 0), stop=False,
                            skip_group_check=True)
                    for dx in range(3):
                        nc.tensor.matmul(
                            ps[0:64, 0:n], wsng_sb[0:64, 3 * li + dx, :],
                            Lp[0:64, oi + GW + dx:oi + GW + dx + n],
                            start=False, stop=(dx == 2),
                            skip_group_check=True)
                        nc.tensor.matmul(
                            ps[64:128, 0:n], wsng_sb[64:128, 3 * li + dx, :],
                            Lp[64:128, oi + dx:oi + dx + n],
                            start=False, stop=(dx == 2),
                            skip_group_check=True)
                    nc.scalar.activation(Li[0:128, o:o + n], ps[0:128, 0:n],
                                         AF.Relu,
                                         bias=b03_sb[:, li + 1:li + 2])
                return Li

            def emit_conv1m(sb):
                st[('L1m', sb)] = emit_convi(f"m{sb}", 0, F1M, "l1", 1,
                                             st.pop(('L0m', sb)))

            def emit_conv23(b, li):
                if li == 1:
                    Lp = st[('L1m', b // 2)]
                    st[('L', b)] = emit_convi(b, 1, F2, "l2", 1, Lp,
                                              ioff=(b % 2) * 8 * GW)
                else:
                    st[('L', b)] = emit_convi(b, 2, F3, "l3", 2,
                                              st.pop(('L', b)))

            def emit_ldup(b):
                L3 = st[('L', b)]
                de = lp.tile([128, F3 + 2], f16, tag="ldupe", bufs=1,
                             name=f"de_{b}")
                do = lp.tile([128, F3 + 2], f16, tag="ldupo", bufs=1,
                             name=f"do_{b}")
                for c0, c1 in ((0, 2 * GW + 378), (2 * GW + 378, 5 * GW),
                               (5 * GW, F3)):
                    nc.sync.dma_start(de[0:64, c0:c1], L3[0:64, c0 + 1:c1 + 1])
                    nc.sync.dma_start(de[64:128, c0:c1],
                                      L3[0:64, c0 + 2:c1 + 2])
                    nc.sync.dma_start(do[0:64, c0:c1],
                                      L3[64:128, c0 + 1:c1 + 1])
                    nc.sync.dma_start(do[64:128, c0:c1],
                                      L3[64:128, c0 + 2:c1 + 2])
                st[('ldup', b)] = (de, do)

            def conv4_sub(b, s):
                L3 = st[('L', b)]
                de, do = st[('ldup', b)]
                E = ep.tile([128, 4, FE], bf16, tag=f"e{s}", bufs=1,
                            name=f"E{b}_{s}")
                for rp in range(4):
                    rho = s * 4 + rp
                    oe = rho * GW
                    oo = (rho + 1) * GW
                    for m in range(4):
                        mm = MBLK[m + 1] - MBLK[m]
                        ms = slice(MBLK[m], MBLK[m + 1])
                        ps4 = ps4p.tile([128, 2, 512], f32, tag="ps4",
                                        name=f"ps4_{b}_{rho}_{m}")
                        # dy-pairs (K=128)
                        for dx in range(3):
                            nc.tensor.matmul(
                                ps4[0:mm, 0, 0:EW], w4pe_sb[:, dx, ms],
                                L3[0:128, oe + dx:oe + dx + EW],
                                start=(dx == 0), stop=False,
                                skip_group_check=True)
                            nc.tensor.matmul(
                                ps4[0:mm, 1, 0:EW], w4po_sb[:, dx, ms],
                                L3[0:128, oo + dx:oo + dx + EW],
                                start=(dx == 0), stop=False,
                                skip_group_check=True)
                        if s == 0 and (rp == 0 or (b == 0 and rp == 1)):
                            # no-dup form: 3 singles per parity, so the
                            # first rows of a block never wait on the
                            # ldup DMA chain
                            for j, wt in ((0, w4s_sb[0:64, ms]),
                                          (1, w4x_sb[0:64, 0, ms]),
                                          (2, w4x_sb[0:64, 1, ms])):
                                nc.tensor.matmul(
                                    ps4[0:mm, 0, 0:EW], wt,
                                    L3[0:64, oo + j:oo + j + EW],
                                    start=False, stop=(j == 2),
                                    skip_group_check=True)
                            for j, wt in ((0, w4s_sb[64:128, ms]),
                                          (1, w4x_sb[64:128, 0, ms]),
                                          (2, w4x_sb[64:128, 1, ms])):
                                nc.tensor.matmul(
                                    ps4[0:mm, 1, 0:EW], wt,
                                    L3[64:128, oe + j:oe + j + EW],
                                    start=False, stop=(j == 2),
                                    skip_group_check=True)
                            nc.scalar.activation(
                                E[0:mm, m, 2 * rp * EW:(2 * rp + 2) * EW],
                                ps4[0:mm, :, 0:EW], AF.Exp,
                                bias=b4_sb[0:mm, m:m + 1])
                            continue
                        # column-shift dup pair (K=128)
                        nc.tensor.matmul(ps4[0:mm, 0, 0:EW],
                                         w4d_sb[:, 0, ms],
                                         de[0:128, oo:oo + EW],
                                         start=False, stop=False,
                                         skip_group_check=True)
                        nc.tensor.matmul(ps4[0:mm, 1, 0:EW],
                                         w4d_sb[:, 1, ms],
                                         do[0:128, oe:oe + EW],
                                         start=False, stop=False,
                                         skip_group_check=True)
                        # the two K=64 singles occupy disjoint row groups
                        nc.tensor.matmul(ps4[0:mm, 0, 0:EW],
                                         w4s_sb[0:64, ms],
                                         L3[0:64, oo:oo + EW],
                                         start=False, stop=True,
                                         skip_group_check=True)
                        nc.tensor.matmul(ps4[0:mm, 1, 0:EW],
                                         w4s_sb[64:128, ms],
                                         L3[64:128, oe:oe + EW],
                                         start=False, stop=True,
                                         skip_group_check=True)
                        nc.scalar.activation(
                            E[0:mm, m, 2 * rp * EW:(2 * rp + 2) * EW],
                            ps4[0:mm, :, 0:EW], AF.Exp,
                            bias=b4_sb[0:mm, m:m + 1])
                return E

            def grp_tile(tag, b, s, ti, grp, E):
                nd = psa.tile([128, 512], f32, tag="psa",
                              name=f"{tag}{b}_{s}_{ti}")
                for m in range(4):
                    mm = MBLK[m + 1] - MBLK[m]
                    for j, (o, n) in enumerate(grp):
                        nc.tensor.matmul(nd[32 * j:32 * j + 8, 0:n],
                                         gm_sb[0:mm, m, :],
                                         E[0:mm, m, o:o + n],
                                         start=(m == 0), stop=(m == 3),
                                         tile_position=(0, 32 * j),
                                         skip_group_check=True)
                return nd

            def den_sub(b, s, E):
                recs = []
                for ti, grp in enumerate(CGROUPS):
                    nd = grp_tile("den", b, s, ti, grp, E)
                    Pn = 32 * (len(grp) - 1) + 8
                    rec = smp.tile([128, 512], f32, tag="rec", bufs=4,
                                   name=f"rec{b}_{s}_{ti}")
                    nc.vector.reciprocal_approx_fast(rec[0:Pn, :],
                                                     nd[0:Pn, 0:512])
                    recs.append(rec)
                return recs

            def mult_sub(b, s, E):
                Prep = st[('prep', b)][s]
                for m in range(4):
                    mm = MBLK[m + 1] - MBLK[m]
                    nc.vector.tensor_mul(E[0:mm, m, 0:MULT_H],
                                         E[0:mm, m, 0:MULT_H],
                                         Prep[0:mm, m, 0:MULT_H])
                    nc.gpsimd.tensor_mul(E[0:mm, m, MULT_H:FE],
                                         E[0:mm, m, MULT_H:FE],
                                         Prep[0:mm, m, MULT_H:FE])

            def num_sub(b, s, E, recs):
                base = (b * RB + s * SRB) * EW
                for ti, grp in enumerate(CGROUPS):
                    nd = grp_tile("num", b, s, ti, grp, E)
                    Pn = 32 * (len(grp) - 1) + 8
                    res = smp.tile([128, 512], f32, tag="res", bufs=2,
                                   name=f"res{b}_{s}_{ti}")
                    nc.vector.tensor_mul(res[0:Pn, :], nd[0:Pn, 0:512],
                                         recs[ti][0:Pn, :])
                    for j, (o, n) in enumerate(grp):
                        nc.sync.dma_start(
                            out.ap()[0:8, base + o:base + o + n],
                            res[32 * j:32 * j + 8, 0:n])

            emit_weights([(w0_sb, w0)])
            emit_x36(0, nchunks=8)
            emit_weights([(b03_sb, b03), (wpe_sb, wpe), (wpo_sb, wpo),
                          (wsng_sb, wsng), (w4pe_sb, w4pe), (w4po_sb, w4po),
                          (w4d_sb, w4d), (w4s_sb, w4s), (w4x_sb, w4x),
                          (b4_sb, b4), (gm_sb, gm)])
            emit_conv0(0)
            emit_prep(0)
            emit_conv1m(0)
            emit_conv23(0, 1)
            emit_conv23(0, 2)
            for b in range(NBLK):
                k = b % 2
                emit_ldup(b)
                if k == 1 and b + 1 < NBLK:
                    emit_x36(b // 2 + 1)
                if b + 1 < NBLK:
                    emit_prep(b + 1)
                E0 = conv4_sub(b, 0)
                recs0 = den_sub(b, 0, E0)
                E1 = conv4_sub(b, 1)
                mult_sub(b, 0, E0)
                recs1 = den_sub(b, 1, E1)
                if k == 1 and b + 1 < NBLK:
                    emit_conv0(b // 2 + 1)
                elif k == 0:
                    emit_conv23(b + 1, 1)
                num_sub(b, 0, E0, recs0)
                mult_sub(b, 1, E1)
                if k == 1 and b + 1 < NBLK:
                    emit_conv1m(b // 2 + 1)
                elif k == 0:
                    emit_conv23(b + 1, 2)
                num_sub(b, 1, E1, recs1)
                if k == 1 and b + 1 < NBLK:
                    emit_conv23(b + 1, 1)
                    emit_conv23(b + 1, 2)

    nc.compile()
    return nc


def _host_prep(inputs):
    mosaic = np.asarray(inputs["mosaic"], dtype=np.float32)
    gray = mosaic.sum(axis=1)                       # [2, 768, 768]
    g0 = gray[:, 0::2, 0::2]
    b_ = gray[:, 1::2, 0::2]
    r = gray[:, 0::2, 1::2]
    g1 = gray[:, 1::2, 1::2]
    x4 = np.stack([g0, b_, r, g1], axis=1)          # [2, 4, 384, 384]
    xpad = np.zeros((BS, 4, QH + 4, XW), dtype=np.float32)
    xpad[:, :, :QH, :QW] = x4

    W0 = np.asarray(inputs["W0"], np.float32)
    w0v = np.zeros((100, 64), np.float32)
    w0flat = W0.transpose(2, 3, 1, 0).reshape(36, 64)
    w0v[0:36] = w0flat
    w0v[64:100] = w0flat

    wpe = np.zeros((128, 9, 64), np.float32)
    wpo = np.zeros((128, 9, 64), np.float32)
    wsng = np.zeros((128, 9, 64), np.float32)
    for li, wname in enumerate(("W1", "W2", "W3")):
        Wi = np.asarray(inputs[wname], np.float32)   # [64out, 64in, 3, 3]
        for dx in range(3):
            wpe[0:64, 3 * li + dx, :] = Wi[:, :, 0, dx].T
            wpe[64:128, 3 * li + dx, :] = Wi[:, :, 1, dx].T
            wpo[0:64, 3 * li + dx, :] = Wi[:, :, 1, dx].T
            wpo[64:128, 3 * li + dx, :] = Wi[:, :, 2, dx].T
            wsng[0:64, 3 * li + dx, :] = Wi[:, :, 2, dx].T
            wsng[64:128, 3 * li + dx, :] = Wi[:, :, 0, dx].T

    W4 = np.asarray(inputs["W4"], np.float32)        # [490, 64, 3, 3]
    w4pe = np.zeros((128, 3, 490), np.float32)
    w4po = np.zeros((128, 3, 490), np.float32)
    w4d = np.zeros((128, 2, 490), np.float32)
    w4s = np.zeros((128, 490), np.float32)
    for dx in range(3):
        w4pe[0:64, dx, :] = W4[:, :, 0, dx].T
        w4pe[64:128, dx, :] = W4[:, :, 1, dx].T
        w4po[0:64, dx, :] = W4[:, :, 1, dx].T
        w4po[64:128, dx, :] = W4[:, :, 2, dx].T
    w4d[0:64, 0, :] = W4[:, :, 2, 1].T
    w4d[64:128, 0, :] = W4[:, :, 2, 2].T
    w4d[0:64, 1, :] = W4[:, :, 0, 1].T
    w4d[64:128, 1, :] = W4[:, :, 0, 2].T
    w4s[0:64, :] = W4[:, :, 2, 0].T
    w4s[64:128, :] = W4[:, :, 0, 0].T
    w4x = np.zeros((128, 2, 490), np.float32)
    w4x[0:64, 0, :] = W4[:, :, 2, 1].T
    w4x[0:64, 1, :] = W4[:, :, 2, 2].T
    w4x[64:128, 0, :] = W4[:, :, 0, 1].T
    w4x[64:128, 1, :] = W4[:, :, 0, 2].T

    b03 = np.zeros((128, 4), np.float32)
    for i in range(4):
        bi = np.asarray(inputs[f"b{i}"], np.float32)
        b03[0:64, i] = bi
        b03[64:128, i] = bi
    b4v = np.asarray(inputs["b4"], np.float32)
    b4p = np.zeros((128, 4), np.float32)
    for c in range(490):
        b4p[c % 128, c // 128] = b4v[c]

    gmk = np.zeros((128, 4, 8), ml_dtypes.bfloat16)
    for c in range(490):
        gmk[c % 128, c // 128, CHUNK_GROUP[c // 49]] = 1

    xpad_bf = xpad.astype(ml_dtypes.bfloat16)
    wcast = {
        "w0": w0v.astype(np.float16),
        "wpe": wpe.astype(np.float16),
        "wpo": wpo.astype(np.float16),
        "wsng": wsng.astype(np.float16),
        "w4pe": w4pe.astype(np.float16),
        "w4po": w4po.astype(np.float16),
        "w4d": w4d.astype(np.float16),
        "w4s": w4s.astype(np.float16),
        "w4x": w4x.astype(np.float16),
        "b03": b03, "b4": b4p, "gm": gmk,
    }
    in_maps = []
    for b in range(BS):
        for band in range(BANDS):
            r0 = band * 94
            slab = np.zeros((4, XR, XW), np.float16)
            hi = min(QH, r0 + XR)
            slab[:, 0:hi - r0, :] = xpad[b, :, r0:hi, :].astype(np.float16)
            # shifted-plane (im2col) tensor for the kernel-apply patches:
            # xg[49*j + 7*dy + dx, jr, jc] = plane_j[r0 + jr + 2 + dy, jc + 2 + dx]
            xgp = np.empty((490, KR, EW), ml_dtypes.bfloat16)
            for j in range(10):
                pl = xpad_bf[b, CHUNK_PLANE[j]]
                for dy in range(KS):
                    for dx in range(KS):
                        c = 49 * j + 7 * dy + dx
                        xgp[c] = pl[r0 + 2 + dy: r0 + 2 + dy + KR,
                                    2 + dx: 2 + dx + EW]
            xi = np.zeros((100, NSB, P0M * GW), np.float16)
            for sb in range(NSB):
                R = sb * 2 * RB
                for dy in range(3):
                    for dx in range(3):
                        for ch in range(4):
                            p = 4 * (3 * dy + dx) + ch
                            ev = slab[ch, R + dy:R + dy + 2 * P0M:2,
                                      dx:dx + GW]
                            od = slab[ch, R + 1 + dy:R + 1 + dy + 2 * P0M:2,
                                      dx:dx + GW]
                            xi[p, sb] = ev.reshape(-1)
                            xi[64 + p, sb] = od.reshape(-1)
            im = {"xs": slab, "xg": xgp, "xi": xi}
            im.update(wcast)
            in_maps.append(im)
    aux = {"g0": g0, "b_": b_, "r": r, "g1": g1}
    return in_maps, aux


def _assemble(results, aux):
    full = np.empty((BS, 3, 2 * KR_TOT, 2 * KR_TOT), np.float32)
    # quarter-res computed planes [8, 374, 374] per batch
    for b in range(BS):
        qs = []
        for band in range(BANDS):
            core = b * BANDS + band
            o = results[core]["out"].reshape(8, KR, EW)
            nvalid = min(94, KR_TOT - band * 94)
            qs.append(o[:, :nvalid, :KR_TOT])
        q = np.concatenate(qs, axis=1)               # [8, 374, 374]
        crop = (slice(5, 5 + KR_TOT), slice(5, 5 + KR_TOT))
        r_pass = aux["r"][b][crop]
        b_pass = aux["b_"][b][crop]
        g0_pass = aux["g0"][b][crop]
        g1_pass = aux["g1"][b][crop]
        # red
        full[b, 0, 0::2, 0::2] = q[0]
        full[b, 0, 0::2, 1::2] = r_pass
        full[b, 0, 1::2, 0::2] = q[1]
        full[b, 0, 1::2, 1::2] = q[2]
        # green
        full[b, 1, 0::2, 0::2] = g0_pass
        full[b, 1, 0::2, 1::2] = q[6]
        full[b, 1, 1::2, 0::2] = q[7]
        full[b, 1, 1::2, 1::2] = g1_pass
        # blue
        full[b, 2, 0::2, 0::2] = q[3]
        full[b, 2, 0::2, 1::2] = q[4]
        full[b, 2, 1::2, 0::2] = b_pass
        full[b, 2, 1::2, 1::2] = q[5]
    return full


def kernel(**inputs):
    global LAST_EXEC_NS, LAST_RESULTS
    from concourse.bass_utils import run_bass_kernel_spmd

    if "nc" not in _cache:
        _cache["nc"] = _build()
    nc = _cache["nc"]

    in_maps, aux = _host_prep(inputs)
    kw = {}
    if TRACE:
        kw["trace"] = True
    res = run_bass_kernel_spmd(nc, in_maps, core_ids=list(range(8)), **kw)
    LAST_EXEC_NS = res.exec_time_ns
    LAST_RESULTS = res
    return _assemble(res.results, aux)
